# revision 35
# baseline (speedup 1.0000x reference)
"""CIN (xDeepFM Compressed Interaction Network) Bass/Tile kernel for TRN2.

Problem: X_0 [1024, 39, 64]; three CIN layers (units 128 each):
    had_i = outer(X_0, X_i) over channel dims, per (b, d)
    X_{i+1} = W_i @ had_i + b_i            (1x1 conv over channels)
    pooled_i = X_{i+1}.sum(d)
Output: concat(pooled_1..3) -> [1024, 384] fp32.

Strategy (pure data-parallel over batch, 8 cores, 128 samples each):
  * channel-major layout on chip: tensors stored [channels, b*64+d]
  * had formation: DVE tensor_mul against 128-row broadcast slabs of X_0
    rows. Even slab groups arrive by partition-broadcast DMA (0-stride
    partition APs over host-permuted contiguous sources; 2048-wide
    super-tiles keep packets at 4-16KB contiguous runs); odd groups are
    generated on the PE via one-hot selector matmuls (esel.T @ X0-block
    broadcasts a row to all partitions) dripped between conv matmuls, so
    slab DMA and PE generation overlap and the PE never idles long
    enough for the HAM clock gate to re-throttle
  * convs: PE matmuls, k-outer loop accumulating 4 x 512-wide PSUM banks
    per super-tile (layer 1: 13 chunks of 117 = 3 h-values x 39 m;
    layer 2: 39 chunks of 128)
  * evacuation: ScalarE Identity activation with per-partition bias;
    pooled_1/pooled_2 via strided DVE d-sum reductions, emitted lazily
    off the critical phase junctions
  * layer 3 never materializes its conv: pooled_3 = W3 @ Gram(X0, X2);
    per 2 samples one PE transpose, then per-sample Gram matmuls with
    output [m, h] so pooled_3 runs as 39 N=128 accumulating matmuls;
    all layer-3 pieces are dripped into the next super-tile's conv loops
  * elementwise dtype fp16 by default (DVE 2x mode, ~3e-4 rel err);
    BASS_CIN_DTYPE=fp32 falls back to full fp32.

Measured on 8 axon TRN2 cores: 520 us HW exec, rel err 3.1e-4
(vs 1348 us for the first working version of the same dataflow).
"""

import os
import numpy as np

import concourse.bass as bass
import concourse.bacc as bacc
import concourse.mybir as mybir
import concourse.tile as tile
from concourse import bass_utils

F32 = mybir.dt.float32
F16 = mybir.dt.float16

B, F, D, U = 1024, 39, 64, 128
NCORES = 8
BC = B // NCORES            # 128 samples per core
BD = BC * D                 # 8192 bd-columns per core
ST = 2048                   # super-tile width (DMA granularity)
NST = BD // ST              # 4
SUB = 512                   # matmul/evac sub-tile width (one PSUM bank)
NSUB = ST // SUB            # 4
SPS = SUB // D              # 8 samples per sub-tile
KG1 = 13                    # layer-1 chunks: 13 x (3 h-values x 39 m) = 117 rows

_CACHE: dict = {}


def _dtype_cfg():
    name = os.environ.get("BASS_CIN_DTYPE", "fp16")
    if name == "fp32":
        return F32, np.float32
    return F16, np.float16


def _off_cfg():
    """How many trailing layer-2 h-groups (of 4) and layer-1 k-groups (of 4)
    per super-tile are generated on the PE (ones-matmul broadcast + ScalarE
    evacuation) instead of DMA."""
    offg2 = int(os.environ.get("BASS_CIN_OFFG2", "4"))
    offg1 = int(os.environ.get("BASS_CIN_OFFG1", "1"))
    return offg2, offg1


def _build(dt_e, offg2, offg1) -> bacc.Bacc:
    nc = bacc.Bacc("TRN2", target_bir_lowering=False, debug=False,
                   enable_asserts=False)
    AF = mybir.ActivationFunctionType

    x0cp_d = nc.dram_tensor("x0cp", [F, BD], dt_e, kind="ExternalInput")
    x0q1_d = nc.dram_tensor("x0q1", [NST, 3, KG1, ST], dt_e, kind="ExternalInput")
    x0q2_d = nc.dram_tensor("x0q2", [NST, F, ST], dt_e, kind="ExternalInput")
    es1_d = nc.dram_tensor("esel1", [F, KG1 * 117], dt_e, kind="ExternalInput")
    es2_d = nc.dram_tensor("esel2", [F, F * U], dt_e, kind="ExternalInput")
    x0dt_d = nc.dram_tensor("x0dt", [D, BC * F], dt_e, kind="ExternalInput")
    w1p_d = nc.dram_tensor("w1p", [117, KG1 * U], dt_e, kind="ExternalInput")
    w2p_d = nc.dram_tensor("w2p", [U, F * U], dt_e, kind="ExternalInput")
    w3p_d = nc.dram_tensor("w3p", [U, F * U], dt_e, kind="ExternalInput")
    b1_d = nc.dram_tensor("b1c", [U, 1], F32, kind="ExternalInput")
    b2_d = nc.dram_tensor("b2c", [U, 1], F32, kind="ExternalInput")
    b3_d = nc.dram_tensor("b3c", [U, 1], F32, kind="ExternalInput")  # 64*b3
    id16_d = nc.dram_tensor("id16", [U, U], dt_e, kind="ExternalInput")
    id32_d = nc.dram_tensor("id32", [U, U], F32, kind="ExternalInput")

    y_d = nc.dram_tensor("y", [BC, 3 * U], F32, kind="ExternalOutput")

    with tile.TileContext(nc) as tc:
        with (
            tc.tile_pool(name="static", bufs=1) as stat,
            tc.tile_pool(name="bc1", bufs=2) as bc1p,
            tc.tile_pool(name="bc2", bufs=3) as bc2p,
            tc.tile_pool(name="had", bufs=4) as hadp,
            tc.tile_pool(name="xsb", bufs=2) as xsbp,
            tc.tile_pool(name="l3sb", bufs=3) as l3p,
            tc.tile_pool(name="ps_conv", bufs=4, space="PSUM") as ppc,
            tc.tile_pool(name="ps_tg", bufs=2, space="PSUM") as pptg,
            tc.tile_pool(name="ps_bc", bufs=2, space="PSUM") as ppbc,
        ):
            # ---- static loads ----
            x0st3 = stat.tile([117, BD], dt_e)          # X0 rows tiled 3x
            for j in range(3):
                nc.sync.dma_start(x0st3[j * F:(j + 1) * F, :], x0cp_d[:, :])
            w1sb = stat.tile([117, KG1 * U], dt_e)
            nc.sync.dma_start(w1sb[:], w1p_d[:, :])
            w2sb = stat.tile([U, F * U], dt_e)
            w3sb = stat.tile([U, F * U], dt_e)
            # two stacked copies (partitions 0-63 and 64-127) so per-sample
            # Gram matmuls can match lhsT base_partition for both halves
            x0dt = stat.tile([2 * D, BC * F], dt_e)
            b1sb = stat.tile([U, 1], F32)
            nc.sync.dma_start(b1sb[:], b1_d[:, :])
            b2sb = stat.tile([U, 1], F32)
            nc.sync.dma_start(b2sb[:], b2_d[:, :])
            b3sb = stat.tile([U, 1], F32)
            nc.sync.dma_start(b3sb[:], b3_d[:, :])
            id16 = stat.tile([U, U], dt_e)
            nc.sync.dma_start(id16[:], id16_d[:, :])
            id32 = stat.tile([U, U], F32)
            nc.sync.dma_start(id32[:], id32_d[:, :])
            esel1 = stat.tile([F, KG1 * 117], dt_e)
            nc.sync.dma_start(esel1[:], es1_d[:, :])
            esel2 = stat.tile([F, F * U], dt_e)

            pooled1 = stat.tile([U, BC], F32)
            pooled2 = stat.tile([U, BC], F32)
            pooled3 = stat.tile([U, BC], F32)
            g2f = stat.tile([U, F, BC], dt_e)           # Gram: [m, h, b]
            outsb = stat.tile([BC, 3 * U], F32)

            # ---- main loop over super-tiles ----
            # fat statics not needed in the first ~30us are dripped during
            # super-tile 0's layer-1 loop so the first broadcast slab DMAs
            # aren't queued behind them on the DGE rings
            l3_pending: list = [
                lambda: nc.sync.dma_start(w2sb[:], w2p_d[:, :]),
                lambda: nc.scalar.dma_start(esel2[:], es2_d[:, :]),
                lambda: nc.sync.dma_start(x0dt[0:D, :], x0dt_d[:, :]),
                lambda: nc.scalar.dma_start(x0dt[D:2 * D, :], x0dt_d[:, :]),
                lambda: nc.scalar.dma_start(w3sb[:], w3p_d[:, :]),
            ]

            def emit_l3():
                if l3_pending:
                    l3_pending.pop(0)()

            for st in range(NST):
                cols = slice(st * ST, (st + 1) * ST)

                # broadcast slabs for this super-tile. Most arrive by
                # partition-broadcast DMA (0-stride partition APs over a
                # contiguous host-permuted source); the trailing offg1/offg2
                # full groups are generated on the PE (ones-matmul -> PSUM)
                # and evacuated by ScalarE, trading DMA for PE+ACT headroom.
                ng1 = (KG1 + 3) // 4                     # 4 k-groups (4,4,4,1)
                ng2 = (F + 3) // 4                       # 10 h-groups (9x4 + 3)
                # interleave PE-built groups between DMA groups so slab DMA
                # transfers and PE broadcast generation run concurrently
                pe1 = set(list(range(1, ng1, 2))[:offg1])
                pe2 = set(list(range(1, ng2, 2))[:offg2])

                def gen_slab_pe2(slab, h, i, sb_i):
                    # slab[:, i, sub] <- broadcast of X0[h, sub-cols] to 128
                    # rows via selector matmul: esel2[:, h-block].T @ x0block.
                    # Evacuations alternate ScalarE/VectorE so neither FIFO
                    # gates the PE through the 2-slot bps pool.
                    c0 = st * ST + sb_i * SUB
                    bps = ppbc.tile([U, SUB], F32, tag="bcps", name="bps")
                    nc.tensor.matmul(
                        bps[:], esel2[:, h * U:(h + 1) * U],
                        x0st3[0:F, c0:c0 + SUB],
                        start=True, stop=True,
                    )
                    dst = slab[:, i, sb_i * SUB:(sb_i + 1) * SUB]
                    if sb_i % 2 == 0:
                        nc.scalar.activation(dst, bps[:], AF.Identity)
                    else:
                        nc.vector.tensor_copy(dst, bps[:])

                # Slab groups are created in strict consumption order with a
                # fixed look-ahead of 2 groups (matches pool bufs=3), whether
                # DMA'd or PE-built. This keeps slot-allocation order equal to
                # consumption order (no FIFO deadlock) and lets slab DMA
                # transfers overlap the PE-built group windows.
                bc1g: dict = {}
                bc2g: dict = {}
                pe1_pieces: dict = {}
                pe_pieces: dict = {}
                dma_rr = [0]

                def ensure_bc1(g):
                    if g >= ng1 or g in bc1g:
                        return
                    k0 = g * 4
                    kcnt = min(4, KG1 - k0)
                    if g in pe1:
                        slab = bc1p.tile([117, 4, ST], dt_e, tag="bc1",
                                         name="bc1pe")
                        pe1_pieces[g] = [(i, sb) for i in range(kcnt)
                                         for sb in range(NSUB)]
                    else:
                        slab = bc1p.tile([117, 4, ST], dt_e, tag="bc1",
                                         name="bc1s")
                        for j in range(3):
                            src = x0q1_d[st, j, k0:k0 + kcnt, :] \
                                .partition_broadcast(F)
                            eng = nc.sync if dma_rr[0] % 2 == 0 else nc.scalar
                            eng.dma_start(
                                slab[j * F:(j + 1) * F, 0:kcnt, :], src)
                        dma_rr[0] += 1
                    bc1g[g] = slab

                def ensure_bc2(g):
                    if g >= ng2 or g in bc2g:
                        return
                    h0 = g * 4
                    hcnt = min(4, F - h0)
                    if g in pe2:
                        slab = bc2p.tile([U, 4, ST], dt_e, tag="bc2",
                                         name="bc2pe")
                        pe_pieces[g] = [(i, sb) for i in range(hcnt)
                                        for sb in range(NSUB)]
                    else:
                        slab = bc2p.tile([U, 4, ST], dt_e, tag="bc2",
                                         name="bc2s")
                        src = x0q2_d[st, h0:h0 + hcnt, :].partition_broadcast(U)
                        eng = nc.sync if dma_rr[0] % 2 == 0 else nc.scalar
                        eng.dma_start(slab[:, 0:hcnt, :], src)
                        dma_rr[0] += 1
                    bc2g[g] = slab

                def gen_slab_pe1(slab, k, i, sb_i):
                    # one selector matmul builds the whole 117-row chunk:
                    # out[p] = X0[3k + p//39]
                    c0 = st * ST + sb_i * SUB
                    bps = ppbc.tile([117, SUB], F32, tag="bcps", name="bps1")
                    nc.tensor.matmul(
                        bps[:], esel1[:, k * 117:(k + 1) * 117],
                        x0st3[0:F, c0:c0 + SUB],
                        start=True, stop=True,
                    )
                    dst = slab[:, i, sb_i * SUB:(sb_i + 1) * SUB]
                    if sb_i % 2 == 0:
                        nc.scalar.activation(dst, bps[:], AF.Identity)
                    else:
                        nc.vector.tensor_copy(dst, bps[:])

                def bc1_drip(k):
                    ensure_bc1(k // 4 + 2)
                    for g in sorted(pe1_pieces):
                        lst = pe1_pieces[g]
                        if lst:
                            for _ in range(4):
                                if not lst:
                                    break
                                i, sb_i2 = lst.pop(0)
                                gen_slab_pe1(bc1g[g], g * 4 + i, i, sb_i2)
                            break

                def bc2_drip(h):
                    ensure_bc2(h // 4 + 2)
                    for g in sorted(pe_pieces):
                        lst = pe_pieces[g]
                        if lst:
                            for _ in range(4):
                                if not lst:
                                    break
                                i, sb_i2 = lst.pop(0)
                                gen_slab_pe2(bc2g[g], g * 4 + i, i, sb_i2)
                            break

                ensure_bc1(0)
                ensure_bc1(1)
                ensure_bc2(0)
                ensure_bc2(1)

                # ---- layer 1: X1 = W1 @ (X0 (x) X0) + b1 ----
                x1ps = [ppc.tile([U, SUB], F32, tag="conv", name=f"x1ps{i}")
                         for i in range(NSUB)]
                for k in range(KG1):
                    had1 = hadp.tile([117, ST], dt_e, tag="had1")
                    nc.vector.tensor_mul(
                        had1[:], x0st3[:, cols], bc1g[k // 4][:, k % 4, :])
                    for sb_i in range(NSUB):
                        nc.tensor.matmul(
                            x1ps[sb_i][:], w1sb[:, k * U:(k + 1) * U],
                            had1[:, sb_i * SUB:(sb_i + 1) * SUB],
                            start=(k == 0), stop=(k == KG1 - 1),
                        )
                    emit_l3()     # drip previous super-tile's layer-3 work
                    bc1_drip(k)
                x1sb = xsbp.tile([U, ST], dt_e, tag="x1")
                for sb_i in range(NSUB):
                    nc.scalar.activation(
                        x1sb[:, sb_i * SUB:(sb_i + 1) * SUB], x1ps[sb_i][:],
                        AF.Identity, bias=b1sb[:], scale=1.0)
                # pooled1 contribution: strided d-sum on DVE, dripped later
                # so it never sits between the layer-1 evacs and layer-2 TTs
                def red1(st=st, x1sb=x1sb):
                    nc.vector.tensor_reduce(
                        pooled1[:, st * (ST // D):(st + 1) * (ST // D)],
                        x1sb[:].rearrange("p (b d) -> p b d", d=D),
                        mybir.AxisListType.X, mybir.AluOpType.add)
                l3_pending.append(red1)

                # ---- layer 2: X2 = W2 @ (X0 (x) X1) + b2 ----
                x2ps = [ppc.tile([U, SUB], F32, tag="conv", name=f"x2ps{i}")
                         for i in range(NSUB)]
                for h in range(F):
                    had2 = hadp.tile([U, ST], dt_e, tag="had2")
                    nc.vector.tensor_mul(
                        had2[:], x1sb[:], bc2g[h // 4][:, h % 4, :])
                    for sb_i in range(NSUB):
                        nc.tensor.matmul(
                            x2ps[sb_i][:], w2sb[:, h * U:(h + 1) * U],
                            had2[:, sb_i * SUB:(sb_i + 1) * SUB],
                            start=(h == 0), stop=(h == F - 1),
                        )
                    emit_l3()
                    bc2_drip(h)
                x2sb = xsbp.tile([U, ST], dt_e, tag="x2")
                for sb_i in range(NSUB):
                    nc.scalar.activation(
                        x2sb[:, sb_i * SUB:(sb_i + 1) * SUB], x2ps[sb_i][:],
                        AF.Identity, bias=b2sb[:], scale=1.0)
                def red2(st=st, x2sb=x2sb):
                    nc.vector.tensor_reduce(
                        pooled2[:, st * (ST // D):(st + 1) * (ST // D)],
                        x2sb[:].rearrange("p (b d) -> p b d", d=D),
                        mybir.AxisListType.X, mybir.AluOpType.add)
                l3_pending.append(red2)

                # ---- layer 3 Gram: G2[m, h, b] = sum_d X2[m,bd] X0[h,bd] ----
                # queue as lazily-emitted pieces, dripped into the next
                # super-tile's conv loops so the PE never starves (HAM warm)
                def queue_l3(st=st, x2sb=x2sb):
                    for s2 in range(ST // D // 2):       # 2 samples / transpose
                        def piece(s2=s2, st=st, x2sb=x2sb):
                            x2t_ps = pptg.tile([U, U], dt_e, tag="tg",
                                               name="x2tps")
                            nc.tensor.transpose(
                                x2t_ps[:],
                                x2sb[:, s2 * 2 * D:(s2 + 1) * 2 * D], id16[:])
                            x2t = l3p.tile([U, U], dt_e, tag="x2t", name="x2t")
                            nc.scalar.activation(x2t[:], x2t_ps[:], AF.Identity)
                            for ls in range(2):
                                b = st * (ST // D) + s2 * 2 + ls
                                g2ps = pptg.tile([U, F], F32, tag="tg",
                                                 name="g2ps")
                                nc.tensor.matmul(
                                    g2ps[:], x2t[ls * D:(ls + 1) * D, :],
                                    x0dt[ls * D:(ls + 1) * D,
                                         b * F:(b + 1) * F],
                                    start=True, stop=True,
                                )
                                nc.scalar.activation(
                                    g2f[:, :, b], g2ps[:], AF.Identity)
                        l3_pending.append(piece)
                queue_l3()
            while l3_pending:     # flush the last super-tile's layer-3 work
                l3_pending.pop(0)()

            # ---- pooled3 = W3 @ G2 + 64*b3 ----
            if True:
                p3ps = ppbc.tile([U, BC], F32, tag="bcps", name="p3ps")
                for h in range(F):
                    nc.tensor.matmul(
                        p3ps[:], w3sb[:, h * U:(h + 1) * U], g2f[:, h, :],
                        start=(h == 0), stop=(h == F - 1),
                    )
                nc.scalar.activation(
                    pooled3[:], p3ps[:], AF.Identity, bias=b3sb[:], scale=1.0)

                # ---- transpose pooled_i -> [b, o] and store ----
                for i, pl in enumerate((pooled1, pooled2, pooled3)):
                    trp = ppbc.tile([BC, U], F32, tag="bcps", name="trp")
                    nc.tensor.transpose(trp[:], pl[:], id32[:])
                    nc.scalar.activation(
                        outsb[:, i * U:(i + 1) * U], trp[:], AF.Identity)
                nc.sync.dma_start(y_d[:, :], outsb[:])

    nc.compile()
    return nc


def _prep_in_maps(inputs, np_e):
    X0 = np.asarray(inputs["X_0"], np.float32)
    W1 = np.asarray(inputs["W1"], np.float32)
    b1 = np.asarray(inputs["b1"], np.float32)
    W2 = np.asarray(inputs["W2"], np.float32)
    b2 = np.asarray(inputs["b2"], np.float32)
    W3 = np.asarray(inputs["W3"], np.float32)
    b3 = np.asarray(inputs["b3"], np.float32)

    # W1 reorder: [117, 13*128]; rows p=j*39+m, cols k*128+o -> W1[o, (3k+j)*39+m]
    w1r = W1.reshape(U, F, F)                    # [o, h, m]
    w1p = np.zeros((117, KG1 * U), np.float32)
    for k in range(KG1):
        for j in range(3):
            w1p[j * F:(j + 1) * F, k * U:(k + 1) * U] = w1r[:, 3 * k + j, :].T
    # [m, h*128+o]
    w2p = W2.reshape(U, F, U).transpose(2, 1, 0).reshape(U, F * U)
    w3p = W3.reshape(U, F, U).transpose(2, 1, 0).reshape(U, F * U)

    # selector matrices for PE-side broadcast generation
    es1 = np.zeros((F, KG1 * 117), np.float32)
    for k in range(KG1):
        for p in range(117):
            es1[3 * k + p // F, k * 117 + p] = 1.0
    es2 = np.zeros((F, F * U), np.float32)
    for h in range(F):
        es2[h, h * U:(h + 1) * U] = 1.0
    es1 = es1.astype(np_e)
    es2 = es2.astype(np_e)

    shared = {
        "w1p": w1p.astype(np_e),
        "w2p": w2p.astype(np_e),
        "w3p": w3p.astype(np_e),
        "b1c": b1.reshape(U, 1).astype(np.float32),
        "b2c": b2.reshape(U, 1).astype(np.float32),
        "b3c": (D * b3).reshape(U, 1).astype(np.float32),
        "id16": np.eye(U, dtype=np_e),
        "id32": np.eye(U, dtype=np.float32),
    }
    in_maps = []
    for c in range(NCORES):
        xs = X0[c * BC:(c + 1) * BC]                         # [128, 39, 64]
        x0cp = xs.transpose(1, 0, 2).reshape(F, BD)          # [h, b*64+d]
        x0dt = xs.transpose(2, 0, 1).reshape(D, BC * F)      # [d, b*39+h]
        x0st = x0cp.reshape(F, NST, ST)
        x0q2 = x0st.transpose(1, 0, 2)                       # [st, h, c]
        x0q1 = np.zeros((NST, 3, KG1, ST), np.float32)
        for j in range(3):
            for k in range(KG1):
                x0q1[:, j, k, :] = x0st[3 * k + j].reshape(NST, ST)
        m = dict(shared)
        m["x0cp"] = x0cp.astype(np_e)
        m["x0dt"] = x0dt.astype(np_e)
        m["x0q1"] = np.ascontiguousarray(x0q1).astype(np_e)
        m["x0q2"] = np.ascontiguousarray(x0q2).astype(np_e)
        m["esel1"] = es1
        m["esel2"] = es2
        in_maps.append(m)
    return in_maps


def _run(inputs, trace=False, **kw):
    dt_e, np_e = _dtype_cfg()
    offg2, offg1 = _off_cfg()
    key = (dt_e, offg2, offg1)
    if key not in _CACHE:
        _CACHE[key] = _build(dt_e, offg2, offg1)
    nc = _CACHE[key]
    in_maps = _prep_in_maps(inputs, np_e)
    res = bass_utils.run_bass_kernel_spmd(
        nc, in_maps, core_ids=list(range(NCORES)), trace=trace, **kw)
    y = np.concatenate([r["y"] for r in res.results], axis=0).astype(np.float32)
    return y, res


def kernel(**inputs) -> np.ndarray:
    y, _ = _run(inputs, trace=False)
    return y


# revision 36
# speedup vs baseline: 1.0153x; 1.0153x over previous
"""CIN (xDeepFM Compressed Interaction Network) Bass/Tile kernel for TRN2.

Problem: X_0 [1024, 39, 64]; three CIN layers (units 128 each):
    had_i = outer(X_0, X_i) over channel dims, per (b, d)
    X_{i+1} = W_i @ had_i + b_i            (1x1 conv over channels)
    pooled_i = X_{i+1}.sum(d)
Output: concat(pooled_1..3) -> [1024, 384] fp32.

Strategy (pure data-parallel over batch, 8 cores, 128 samples each):
  * channel-major layout on chip: tensors stored [channels, b*64+d]
  * had formation: DVE tensor_mul against 128-row broadcast slabs of X_0
    rows. Even slab groups arrive by partition-broadcast DMA (0-stride
    partition APs over host-permuted contiguous sources; 2048-wide
    super-tiles keep packets at 4-16KB contiguous runs); odd groups are
    generated on the PE via one-hot selector matmuls (esel.T @ X0-block
    broadcasts a row to all partitions) dripped between conv matmuls, so
    slab DMA and PE generation overlap and the PE never idles long
    enough for the HAM clock gate to re-throttle
  * convs: PE matmuls, k-outer loop accumulating 4 x 512-wide PSUM banks
    per super-tile (layer 1: 13 chunks of 117 = 3 h-values x 39 m;
    layer 2: 39 chunks of 128)
  * evacuation: ScalarE Identity activation with per-partition bias;
    pooled_1/pooled_2 via strided DVE d-sum reductions, emitted lazily
    off the critical phase junctions
  * layer 3 never materializes its conv: pooled_3 = W3 @ Gram(X0, X2);
    per 2 samples one PE transpose, then per-sample Gram matmuls with
    output [m, h] so pooled_3 runs as 39 N=128 accumulating matmuls;
    all layer-3 pieces are dripped into the next super-tile's conv loops
  * elementwise dtype fp16 by default (DVE 2x mode, ~3e-4 rel err);
    BASS_CIN_DTYPE=fp32 falls back to full fp32.

Measured on 8 axon TRN2 cores: 520 us HW exec, rel err 3.1e-4
(vs 1348 us for the first working version of the same dataflow).
"""

import os
import numpy as np

import concourse.bass as bass
import concourse.bacc as bacc
import concourse.mybir as mybir
import concourse.tile as tile
from concourse import bass_utils

F32 = mybir.dt.float32
F16 = mybir.dt.float16

B, F, D, U = 1024, 39, 64, 128
NCORES = 8
BC = B // NCORES            # 128 samples per core
BD = BC * D                 # 8192 bd-columns per core
ST = 2048                   # super-tile width (DMA granularity)
NST = BD // ST              # 4
SUB = 512                   # matmul/evac sub-tile width (one PSUM bank)
NSUB = ST // SUB            # 4
SPS = SUB // D              # 8 samples per sub-tile
KG1 = 13                    # layer-1 chunks: 13 x (3 h-values x 39 m) = 117 rows

_CACHE: dict = {}


def _dtype_cfg():
    name = os.environ.get("BASS_CIN_DTYPE", "fp16")
    if name == "fp32":
        return F32, np.float32
    return F16, np.float16


def _off_cfg():
    """How many trailing layer-2 h-groups (of 4) and layer-1 k-groups (of 4)
    per super-tile are generated on the PE (ones-matmul broadcast + ScalarE
    evacuation) instead of DMA."""
    offg2 = int(os.environ.get("BASS_CIN_OFFG2", "4"))
    offg1 = int(os.environ.get("BASS_CIN_OFFG1", "1"))
    return offg2, offg1


def _build(dt_e, offg2, offg1) -> bacc.Bacc:
    nc = bacc.Bacc("TRN2", target_bir_lowering=False, debug=False,
                   enable_asserts=False)
    AF = mybir.ActivationFunctionType

    x0cp_d = nc.dram_tensor("x0cp", [F, BD], dt_e, kind="ExternalInput")
    x0q1_d = nc.dram_tensor("x0q1", [NST, 3, KG1, ST], dt_e, kind="ExternalInput")
    x0q2_d = nc.dram_tensor("x0q2", [NST, F, ST], dt_e, kind="ExternalInput")
    es1_d = nc.dram_tensor("esel1", [F, KG1 * 117], dt_e, kind="ExternalInput")
    es2_d = nc.dram_tensor("esel2", [F, F * U], dt_e, kind="ExternalInput")
    x0dt_d = nc.dram_tensor("x0dt", [D, BC * F], dt_e, kind="ExternalInput")
    w1p_d = nc.dram_tensor("w1p", [117, KG1 * U], dt_e, kind="ExternalInput")
    w2p_d = nc.dram_tensor("w2p", [U, F * U], dt_e, kind="ExternalInput")
    w3p_d = nc.dram_tensor("w3p", [U, F * U], dt_e, kind="ExternalInput")
    b1_d = nc.dram_tensor("b1c", [U, 1], F32, kind="ExternalInput")
    b2_d = nc.dram_tensor("b2c", [U, 1], F32, kind="ExternalInput")
    b3_d = nc.dram_tensor("b3c", [U, 1], F32, kind="ExternalInput")  # 64*b3
    id16_d = nc.dram_tensor("id16", [U, U], dt_e, kind="ExternalInput")
    id32_d = nc.dram_tensor("id32", [U, U], F32, kind="ExternalInput")

    y_d = nc.dram_tensor("y", [BC, 3 * U], F32, kind="ExternalOutput")

    with tile.TileContext(nc) as tc:
        with (
            tc.tile_pool(name="static", bufs=1) as stat,
            tc.tile_pool(name="bc1", bufs=2) as bc1p,
            tc.tile_pool(name="bc2", bufs=3) as bc2p,
            tc.tile_pool(name="had", bufs=4) as hadp,
            tc.tile_pool(name="xsb", bufs=2) as xsbp,
            tc.tile_pool(name="l3sb", bufs=3) as l3p,
            tc.tile_pool(name="ps_conv", bufs=4, space="PSUM") as ppc,
            tc.tile_pool(name="ps_tg", bufs=2, space="PSUM") as pptg,
            tc.tile_pool(name="ps_bc", bufs=2, space="PSUM") as ppbc,
        ):
            # ---- static loads ----
            x0st3 = stat.tile([117, BD], dt_e)          # X0 rows tiled 3x
            for j in range(3):
                nc.sync.dma_start(x0st3[j * F:(j + 1) * F, :], x0cp_d[:, :])
            w1sb = stat.tile([117, KG1 * U], dt_e)
            nc.sync.dma_start(w1sb[:], w1p_d[:, :])
            w2sb = stat.tile([U, F * U], dt_e)
            nc.sync.dma_start(w2sb[:], w2p_d[:, :])
            w3sb = stat.tile([U, F * U], dt_e)
            nc.sync.dma_start(w3sb[:], w3p_d[:, :])
            # two stacked copies (partitions 0-63 and 64-127) so per-sample
            # Gram matmuls can match lhsT base_partition for both halves
            x0dt = stat.tile([2 * D, BC * F], dt_e)
            nc.sync.dma_start(x0dt[0:D, :], x0dt_d[:, :])
            nc.sync.dma_start(x0dt[D:2 * D, :], x0dt_d[:, :])
            b1sb = stat.tile([U, 1], F32)
            nc.sync.dma_start(b1sb[:], b1_d[:, :])
            b2sb = stat.tile([U, 1], F32)
            nc.sync.dma_start(b2sb[:], b2_d[:, :])
            b3sb = stat.tile([U, 1], F32)
            nc.sync.dma_start(b3sb[:], b3_d[:, :])
            id16 = stat.tile([U, U], dt_e)
            nc.sync.dma_start(id16[:], id16_d[:, :])
            id32 = stat.tile([U, U], F32)
            nc.sync.dma_start(id32[:], id32_d[:, :])
            esel1 = stat.tile([F, KG1 * 117], dt_e)
            nc.sync.dma_start(esel1[:], es1_d[:, :])
            esel2 = stat.tile([F, F * U], dt_e)
            nc.sync.dma_start(esel2[:], es2_d[:, :])

            pooled1 = stat.tile([U, BC], F32)
            pooled2 = stat.tile([U, BC], F32)
            pooled3 = stat.tile([U, BC], F32)
            g2f = stat.tile([U, F, BC], dt_e)           # Gram: [m, h, b]
            outsb = stat.tile([BC, 3 * U], F32)

            # ---- main loop over super-tiles ----
            l3_pending: list = []

            def emit_l3():
                if l3_pending:
                    l3_pending.pop(0)()

            for st in range(NST):
                cols = slice(st * ST, (st + 1) * ST)

                # broadcast slabs for this super-tile. Most arrive by
                # partition-broadcast DMA (0-stride partition APs over a
                # contiguous host-permuted source); the trailing offg1/offg2
                # full groups are generated on the PE (ones-matmul -> PSUM)
                # and evacuated by ScalarE, trading DMA for PE+ACT headroom.
                ng1 = (KG1 + 3) // 4                     # 4 k-groups (4,4,4,1)
                ng2 = (F + 3) // 4                       # 10 h-groups (9x4 + 3)
                # interleave PE-built groups between DMA groups so slab DMA
                # transfers and PE broadcast generation run concurrently
                pe1 = set(list(range(1, ng1, 2))[:offg1])
                pe2 = set(list(range(1, ng2, 2))[:offg2])

                def gen_slab_pe2(slab, h, i, sb_i):
                    # slab[:, i, sub] <- broadcast of X0[h, sub-cols] to 128
                    # rows via selector matmul: esel2[:, h-block].T @ x0block.
                    # Evacuations alternate ScalarE/VectorE so neither FIFO
                    # gates the PE through the 2-slot bps pool.
                    c0 = st * ST + sb_i * SUB
                    bps = ppbc.tile([U, SUB], F32, tag="bcps", name="bps")
                    nc.tensor.matmul(
                        bps[:], esel2[:, h * U:(h + 1) * U],
                        x0st3[0:F, c0:c0 + SUB],
                        start=True, stop=True,
                    )
                    dst = slab[:, i, sb_i * SUB:(sb_i + 1) * SUB]
                    if sb_i % 2 == 0:
                        nc.scalar.activation(dst, bps[:], AF.Identity)
                    else:
                        nc.vector.tensor_copy(dst, bps[:])

                # Slab groups are created in strict consumption order with a
                # fixed look-ahead of 2 groups (matches pool bufs=3), whether
                # DMA'd or PE-built. This keeps slot-allocation order equal to
                # consumption order (no FIFO deadlock) and lets slab DMA
                # transfers overlap the PE-built group windows.
                bc1g: dict = {}
                bc2g: dict = {}
                pe1_pieces: dict = {}
                pe_pieces: dict = {}
                dma_rr = [0]

                def ensure_bc1(g):
                    if g >= ng1 or g in bc1g:
                        return
                    k0 = g * 4
                    kcnt = min(4, KG1 - k0)
                    if g in pe1:
                        slab = bc1p.tile([117, 4, ST], dt_e, tag="bc1",
                                         name="bc1pe")
                        pe1_pieces[g] = [(i, sb) for i in range(kcnt)
                                         for sb in range(NSUB)]
                    else:
                        slab = bc1p.tile([117, 4, ST], dt_e, tag="bc1",
                                         name="bc1s")
                        for j in range(3):
                            src = x0q1_d[st, j, k0:k0 + kcnt, :] \
                                .partition_broadcast(F)
                            eng = nc.sync if dma_rr[0] % 2 == 0 else nc.scalar
                            eng.dma_start(
                                slab[j * F:(j + 1) * F, 0:kcnt, :], src)
                        dma_rr[0] += 1
                    bc1g[g] = slab

                def ensure_bc2(g):
                    if g >= ng2 or g in bc2g:
                        return
                    h0 = g * 4
                    hcnt = min(4, F - h0)
                    if g in pe2:
                        slab = bc2p.tile([U, 4, ST], dt_e, tag="bc2",
                                         name="bc2pe")
                        pe_pieces[g] = [(i, sb) for i in range(hcnt)
                                        for sb in range(NSUB)]
                    else:
                        slab = bc2p.tile([U, 4, ST], dt_e, tag="bc2",
                                         name="bc2s")
                        src = x0q2_d[st, h0:h0 + hcnt, :].partition_broadcast(U)
                        eng = nc.sync if dma_rr[0] % 2 == 0 else nc.scalar
                        eng.dma_start(slab[:, 0:hcnt, :], src)
                        dma_rr[0] += 1
                    bc2g[g] = slab

                def gen_slab_pe1(slab, k, i, sb_i):
                    # one selector matmul builds the whole 117-row chunk:
                    # out[p] = X0[3k + p//39]
                    c0 = st * ST + sb_i * SUB
                    bps = ppbc.tile([117, SUB], F32, tag="bcps", name="bps1")
                    nc.tensor.matmul(
                        bps[:], esel1[:, k * 117:(k + 1) * 117],
                        x0st3[0:F, c0:c0 + SUB],
                        start=True, stop=True,
                    )
                    dst = slab[:, i, sb_i * SUB:(sb_i + 1) * SUB]
                    if sb_i % 2 == 0:
                        nc.scalar.activation(dst, bps[:], AF.Identity)
                    else:
                        nc.vector.tensor_copy(dst, bps[:])

                def bc1_drip(k):
                    ensure_bc1(k // 4 + 2)
                    for g in sorted(pe1_pieces):
                        lst = pe1_pieces[g]
                        if lst:
                            for _ in range(4):
                                if not lst:
                                    break
                                i, sb_i2 = lst.pop(0)
                                gen_slab_pe1(bc1g[g], g * 4 + i, i, sb_i2)
                            break

                def bc2_drip(h):
                    ensure_bc2(h // 4 + 2)
                    for g in sorted(pe_pieces):
                        lst = pe_pieces[g]
                        if lst:
                            for _ in range(4):
                                if not lst:
                                    break
                                i, sb_i2 = lst.pop(0)
                                gen_slab_pe2(bc2g[g], g * 4 + i, i, sb_i2)
                            break

                ensure_bc1(0)
                ensure_bc1(1)
                ensure_bc2(0)
                ensure_bc2(1)

                # ---- layer 1: X1 = W1 @ (X0 (x) X0) + b1 ----
                x1ps = [ppc.tile([U, SUB], F32, tag="conv", name=f"x1ps{i}")
                         for i in range(NSUB)]
                for k in range(KG1):
                    had1 = hadp.tile([117, ST], dt_e, tag="had1")
                    nc.vector.tensor_mul(
                        had1[:], x0st3[:, cols], bc1g[k // 4][:, k % 4, :])
                    for sb_i in range(NSUB):
                        nc.tensor.matmul(
                            x1ps[sb_i][:], w1sb[:, k * U:(k + 1) * U],
                            had1[:, sb_i * SUB:(sb_i + 1) * SUB],
                            start=(k == 0), stop=(k == KG1 - 1),
                        )
                    emit_l3()     # drip previous super-tile's layer-3 work
                    bc1_drip(k)
                x1sb = xsbp.tile([U, ST], dt_e, tag="x1")
                for sb_i in range(NSUB):
                    nc.scalar.activation(
                        x1sb[:, sb_i * SUB:(sb_i + 1) * SUB], x1ps[sb_i][:],
                        AF.Identity, bias=b1sb[:], scale=1.0)
                # pooled1 contribution: strided d-sum on DVE, dripped later
                # so it never sits between the layer-1 evacs and layer-2 TTs
                def red1(st=st, x1sb=x1sb):
                    nc.vector.tensor_reduce(
                        pooled1[:, st * (ST // D):(st + 1) * (ST // D)],
                        x1sb[:].rearrange("p (b d) -> p b d", d=D),
                        mybir.AxisListType.X, mybir.AluOpType.add)
                l3_pending.append(red1)

                # ---- layer 2: X2 = W2 @ (X0 (x) X1) + b2 ----
                x2ps = [ppc.tile([U, SUB], F32, tag="conv", name=f"x2ps{i}")
                         for i in range(NSUB)]
                for h in range(F):
                    had2 = hadp.tile([U, ST], dt_e, tag="had2")
                    nc.vector.tensor_mul(
                        had2[:], x1sb[:], bc2g[h // 4][:, h % 4, :])
                    for sb_i in range(NSUB):
                        nc.tensor.matmul(
                            x2ps[sb_i][:], w2sb[:, h * U:(h + 1) * U],
                            had2[:, sb_i * SUB:(sb_i + 1) * SUB],
                            start=(h == 0), stop=(h == F - 1),
                        )
                    emit_l3()
                    bc2_drip(h)
                x2sb = xsbp.tile([U, ST], dt_e, tag="x2")
                for sb_i in range(NSUB):
                    nc.scalar.activation(
                        x2sb[:, sb_i * SUB:(sb_i + 1) * SUB], x2ps[sb_i][:],
                        AF.Identity, bias=b2sb[:], scale=1.0)
                def red2(st=st, x2sb=x2sb):
                    nc.vector.tensor_reduce(
                        pooled2[:, st * (ST // D):(st + 1) * (ST // D)],
                        x2sb[:].rearrange("p (b d) -> p b d", d=D),
                        mybir.AxisListType.X, mybir.AluOpType.add)
                l3_pending.append(red2)

                # ---- layer 3 Gram: G2[m, h, b] = sum_d X2[m,bd] X0[h,bd] ----
                # queue as lazily-emitted pieces, dripped into the next
                # super-tile's conv loops so the PE never starves (HAM warm)
                def queue_l3(st=st, x2sb=x2sb):
                    for s2 in range(ST // D // 2):       # 2 samples / transpose
                        def piece(s2=s2, st=st, x2sb=x2sb):
                            x2t_ps = pptg.tile([U, U], dt_e, tag="tg",
                                               name="x2tps")
                            nc.tensor.transpose(
                                x2t_ps[:],
                                x2sb[:, s2 * 2 * D:(s2 + 1) * 2 * D], id16[:])
                            x2t = l3p.tile([U, U], dt_e, tag="x2t", name="x2t")
                            nc.scalar.activation(x2t[:], x2t_ps[:], AF.Identity)
                            for ls in range(2):
                                b = st * (ST // D) + s2 * 2 + ls
                                g2ps = pptg.tile([U, F], F32, tag="tg",
                                                 name="g2ps")
                                nc.tensor.matmul(
                                    g2ps[:], x2t[ls * D:(ls + 1) * D, :],
                                    x0dt[ls * D:(ls + 1) * D,
                                         b * F:(b + 1) * F],
                                    start=True, stop=True,
                                )
                                nc.scalar.activation(
                                    g2f[:, :, b], g2ps[:], AF.Identity)
                        l3_pending.append(piece)
                queue_l3()
            while l3_pending:     # flush the last super-tile's layer-3 work
                l3_pending.pop(0)()

            # ---- pooled3 = W3 @ G2 + 64*b3 ----
            if True:
                p3ps = ppbc.tile([U, BC], F32, tag="bcps", name="p3ps")
                for h in range(F):
                    nc.tensor.matmul(
                        p3ps[:], w3sb[:, h * U:(h + 1) * U], g2f[:, h, :],
                        start=(h == 0), stop=(h == F - 1),
                    )
                nc.scalar.activation(
                    pooled3[:], p3ps[:], AF.Identity, bias=b3sb[:], scale=1.0)

                # ---- transpose pooled_i -> [b, o] and store ----
                for i, pl in enumerate((pooled1, pooled2, pooled3)):
                    trp = ppbc.tile([BC, U], F32, tag="bcps", name="trp")
                    nc.tensor.transpose(trp[:], pl[:], id32[:])
                    nc.scalar.activation(
                        outsb[:, i * U:(i + 1) * U], trp[:], AF.Identity)
                nc.sync.dma_start(y_d[:, :], outsb[:])

    nc.compile()
    return nc


def _prep_in_maps(inputs, np_e):
    X0 = np.asarray(inputs["X_0"], np.float32)
    W1 = np.asarray(inputs["W1"], np.float32)
    b1 = np.asarray(inputs["b1"], np.float32)
    W2 = np.asarray(inputs["W2"], np.float32)
    b2 = np.asarray(inputs["b2"], np.float32)
    W3 = np.asarray(inputs["W3"], np.float32)
    b3 = np.asarray(inputs["b3"], np.float32)

    # W1 reorder: [117, 13*128]; rows p=j*39+m, cols k*128+o -> W1[o, (3k+j)*39+m]
    w1r = W1.reshape(U, F, F)                    # [o, h, m]
    w1p = np.zeros((117, KG1 * U), np.float32)
    for k in range(KG1):
        for j in range(3):
            w1p[j * F:(j + 1) * F, k * U:(k + 1) * U] = w1r[:, 3 * k + j, :].T
    # [m, h*128+o]
    w2p = W2.reshape(U, F, U).transpose(2, 1, 0).reshape(U, F * U)
    w3p = W3.reshape(U, F, U).transpose(2, 1, 0).reshape(U, F * U)

    # selector matrices for PE-side broadcast generation
    es1 = np.zeros((F, KG1 * 117), np.float32)
    for k in range(KG1):
        for p in range(117):
            es1[3 * k + p // F, k * 117 + p] = 1.0
    es2 = np.zeros((F, F * U), np.float32)
    for h in range(F):
        es2[h, h * U:(h + 1) * U] = 1.0
    es1 = es1.astype(np_e)
    es2 = es2.astype(np_e)

    shared = {
        "w1p": w1p.astype(np_e),
        "w2p": w2p.astype(np_e),
        "w3p": w3p.astype(np_e),
        "b1c": b1.reshape(U, 1).astype(np.float32),
        "b2c": b2.reshape(U, 1).astype(np.float32),
        "b3c": (D * b3).reshape(U, 1).astype(np.float32),
        "id16": np.eye(U, dtype=np_e),
        "id32": np.eye(U, dtype=np.float32),
    }
    in_maps = []
    for c in range(NCORES):
        xs = X0[c * BC:(c + 1) * BC]                         # [128, 39, 64]
        x0cp = xs.transpose(1, 0, 2).reshape(F, BD)          # [h, b*64+d]
        x0dt = xs.transpose(2, 0, 1).reshape(D, BC * F)      # [d, b*39+h]
        x0st = x0cp.reshape(F, NST, ST)
        x0q2 = x0st.transpose(1, 0, 2)                       # [st, h, c]
        x0q1 = np.zeros((NST, 3, KG1, ST), np.float32)
        for j in range(3):
            for k in range(KG1):
                x0q1[:, j, k, :] = x0st[3 * k + j].reshape(NST, ST)
        m = dict(shared)
        m["x0cp"] = x0cp.astype(np_e)
        m["x0dt"] = x0dt.astype(np_e)
        m["x0q1"] = np.ascontiguousarray(x0q1).astype(np_e)
        m["x0q2"] = np.ascontiguousarray(x0q2).astype(np_e)
        m["esel1"] = es1
        m["esel2"] = es2
        in_maps.append(m)
    return in_maps


def _run(inputs, trace=False, **kw):
    dt_e, np_e = _dtype_cfg()
    offg2, offg1 = _off_cfg()
    key = (dt_e, offg2, offg1)
    if key not in _CACHE:
        _CACHE[key] = _build(dt_e, offg2, offg1)
    nc = _CACHE[key]
    in_maps = _prep_in_maps(inputs, np_e)
    res = bass_utils.run_bass_kernel_spmd(
        nc, in_maps, core_ids=list(range(NCORES)), trace=trace, **kw)
    y = np.concatenate([r["y"] for r in res.results], axis=0).astype(np.float32)
    return y, res


def kernel(**inputs) -> np.ndarray:
    y, _ = _run(inputs, trace=False)
    return y


# revision 37
# speedup vs baseline: 1.0303x; 1.0148x over previous
"""CIN (xDeepFM Compressed Interaction Network) Bass/Tile kernel for TRN2.

Problem: X_0 [1024, 39, 64]; three CIN layers (units 128 each):
    had_i = outer(X_0, X_i) over channel dims, per (b, d)
    X_{i+1} = W_i @ had_i + b_i            (1x1 conv over channels)
    pooled_i = X_{i+1}.sum(d)
Output: concat(pooled_1..3) -> [1024, 384] fp32.

Strategy (pure data-parallel over batch, 8 cores, 128 samples each):
  * channel-major layout on chip: tensors stored [channels, b*64+d]
  * had formation: DVE tensor_mul against 128-row broadcast slabs of X_0
    rows. Even slab groups arrive by partition-broadcast DMA (0-stride
    partition APs over host-permuted contiguous sources; 2048-wide
    super-tiles keep packets at 4-16KB contiguous runs); odd groups are
    generated on the PE via one-hot selector matmuls (esel.T @ X0-block
    broadcasts a row to all partitions) dripped between conv matmuls, so
    slab DMA and PE generation overlap and the PE never idles long
    enough for the HAM clock gate to re-throttle
  * convs: PE matmuls, k-outer loop accumulating 4 x 512-wide PSUM banks
    per super-tile (layer 1: 13 chunks of 117 = 3 h-values x 39 m;
    layer 2: 39 chunks of 128)
  * evacuation: ScalarE Identity activation with per-partition bias;
    pooled_1/pooled_2 via strided DVE d-sum reductions, emitted lazily
    off the critical phase junctions
  * layer 3 never materializes its conv: pooled_3 = W3 @ Gram(X0, X2);
    per 2 samples one PE transpose, then per-sample Gram matmuls with
    output [m, h] so pooled_3 runs as 39 N=128 accumulating matmuls;
    all layer-3 pieces are dripped into the next super-tile's conv loops
  * elementwise dtype fp16 by default (DVE 2x mode, ~3e-4 rel err);
    BASS_CIN_DTYPE=fp32 falls back to full fp32.

Measured on 8 axon TRN2 cores: 520 us HW exec, rel err 3.1e-4
(vs 1348 us for the first working version of the same dataflow).
"""

import os
import numpy as np

import concourse.bass as bass
import concourse.bacc as bacc
import concourse.mybir as mybir
import concourse.tile as tile
from concourse import bass_utils

F32 = mybir.dt.float32
F16 = mybir.dt.float16

B, F, D, U = 1024, 39, 64, 128
NCORES = 8
BC = B // NCORES            # 128 samples per core
BD = BC * D                 # 8192 bd-columns per core
ST = 2048                   # super-tile width (DMA granularity)
NST = BD // ST              # 4
SUB = 512                   # matmul/evac sub-tile width (one PSUM bank)
NSUB = ST // SUB            # 4
SPS = SUB // D              # 8 samples per sub-tile
KG1 = 13                    # layer-1 chunks: 13 x (3 h-values x 39 m) = 117 rows

_CACHE: dict = {}


def _dtype_cfg():
    name = os.environ.get("BASS_CIN_DTYPE", "fp16")
    if name == "fp32":
        return F32, np.float32
    return F16, np.float16


def _off_cfg():
    """How many trailing layer-2 h-groups (of 4) and layer-1 k-groups (of 4)
    per super-tile are generated on the PE (ones-matmul broadcast + ScalarE
    evacuation) instead of DMA."""
    offg2 = int(os.environ.get("BASS_CIN_OFFG2", "4"))
    offg1 = int(os.environ.get("BASS_CIN_OFFG1", "1"))
    return offg2, offg1


def _build(dt_e, offg2, offg1) -> bacc.Bacc:
    nc = bacc.Bacc("TRN2", target_bir_lowering=False, debug=False,
                   enable_asserts=False)
    AF = mybir.ActivationFunctionType

    x0cp_d = nc.dram_tensor("x0cp", [F, BD], dt_e, kind="ExternalInput")
    x0q1_d = nc.dram_tensor("x0q1", [NST, 3, KG1, ST], dt_e, kind="ExternalInput")
    x0q2_d = nc.dram_tensor("x0q2", [NST, F, ST], dt_e, kind="ExternalInput")
    es1_d = nc.dram_tensor("esel1", [F, KG1 * 117], dt_e, kind="ExternalInput")
    es2_d = nc.dram_tensor("esel2", [F, F * U], dt_e, kind="ExternalInput")
    x0dt_d = nc.dram_tensor("x0dt", [D, BC * F], dt_e, kind="ExternalInput")
    w1p_d = nc.dram_tensor("w1p", [117, KG1 * U], dt_e, kind="ExternalInput")
    w2p_d = nc.dram_tensor("w2p", [U, F * U], dt_e, kind="ExternalInput")
    w3p_d = nc.dram_tensor("w3p", [U, F * U], dt_e, kind="ExternalInput")
    b1_d = nc.dram_tensor("b1c", [U, 1], F32, kind="ExternalInput")
    b2_d = nc.dram_tensor("b2c", [U, 1], F32, kind="ExternalInput")
    b3_d = nc.dram_tensor("b3c", [U, 1], F32, kind="ExternalInput")  # 64*b3
    id16_d = nc.dram_tensor("id16", [U, U], dt_e, kind="ExternalInput")
    id32_d = nc.dram_tensor("id32", [U, U], F32, kind="ExternalInput")

    y_d = nc.dram_tensor("y", [BC, 3 * U], F32, kind="ExternalOutput")

    with tile.TileContext(nc) as tc:
        with (
            tc.tile_pool(name="static", bufs=1) as stat,
            tc.tile_pool(name="bc1", bufs=2) as bc1p,
            tc.tile_pool(name="bc2", bufs=3) as bc2p,
            tc.tile_pool(name="had", bufs=4) as hadp,
            tc.tile_pool(name="xsb", bufs=2) as xsbp,
            tc.tile_pool(name="l3sb", bufs=3) as l3p,
            tc.tile_pool(name="ps_conv", bufs=4, space="PSUM") as ppc,
            tc.tile_pool(name="ps_tg", bufs=2, space="PSUM") as pptg,
            tc.tile_pool(name="ps_bc", bufs=2, space="PSUM") as ppbc,
        ):
            # ---- static loads ----
            x0st3 = stat.tile([117, BD], dt_e)          # X0 rows tiled 3x
            for j in range(3):
                nc.sync.dma_start(x0st3[j * F:(j + 1) * F, :], x0cp_d[:, :])
            w1sb = stat.tile([117, KG1 * U], dt_e)
            nc.sync.dma_start(w1sb[:], w1p_d[:, :])
            w2sb = stat.tile([U, F * U], dt_e)
            nc.gpsimd.dma_start(w2sb[:], w2p_d[:, :])
            w3sb = stat.tile([U, F * U], dt_e)
            nc.gpsimd.dma_start(w3sb[:], w3p_d[:, :])
            # two stacked copies (partitions 0-63 and 64-127) so per-sample
            # Gram matmuls can match lhsT base_partition for both halves
            x0dt = stat.tile([2 * D, BC * F], dt_e)
            nc.gpsimd.dma_start(x0dt[0:D, :], x0dt_d[:, :])
            nc.gpsimd.dma_start(x0dt[D:2 * D, :], x0dt_d[:, :])
            b1sb = stat.tile([U, 1], F32)
            nc.sync.dma_start(b1sb[:], b1_d[:, :])
            b2sb = stat.tile([U, 1], F32)
            nc.sync.dma_start(b2sb[:], b2_d[:, :])
            b3sb = stat.tile([U, 1], F32)
            nc.sync.dma_start(b3sb[:], b3_d[:, :])
            id16 = stat.tile([U, U], dt_e)
            nc.gpsimd.dma_start(id16[:], id16_d[:, :])
            id32 = stat.tile([U, U], F32)
            nc.gpsimd.dma_start(id32[:], id32_d[:, :])
            esel1 = stat.tile([F, KG1 * 117], dt_e)
            nc.sync.dma_start(esel1[:], es1_d[:, :])
            esel2 = stat.tile([F, F * U], dt_e)
            nc.gpsimd.dma_start(esel2[:], es2_d[:, :])

            b1bc = b1sb[:].broadcast_to((U, SUB))
            b2bc = b2sb[:].broadcast_to((U, SUB))
            pooled1 = stat.tile([U, BC], F32)
            pooled2 = stat.tile([U, BC], F32)
            pooled3 = stat.tile([U, BC], F32)
            g2f = stat.tile([U, F, BC], dt_e)           # Gram: [m, h, b]
            outsb = stat.tile([BC, 3 * U], F32)

            # ---- main loop over super-tiles ----
            l3_pending: list = []

            def emit_l3():
                if l3_pending:
                    l3_pending.pop(0)()

            for st in range(NST):
                cols = slice(st * ST, (st + 1) * ST)

                # broadcast slabs for this super-tile. Most arrive by
                # partition-broadcast DMA (0-stride partition APs over a
                # contiguous host-permuted source); the trailing offg1/offg2
                # full groups are generated on the PE (ones-matmul -> PSUM)
                # and evacuated by ScalarE, trading DMA for PE+ACT headroom.
                ng1 = (KG1 + 3) // 4                     # 4 k-groups (4,4,4,1)
                ng2 = (F + 3) // 4                       # 10 h-groups (9x4 + 3)
                # interleave PE-built groups between DMA groups so slab DMA
                # transfers and PE broadcast generation run concurrently
                pe1 = set(list(range(1, ng1, 2))[:offg1])
                pe2 = set(list(range(1, ng2, 2))[:offg2])

                def gen_slab_pe2(slab, h, i, sb_i):
                    # slab[:, i, sub] <- broadcast of X0[h, sub-cols] to 128
                    # rows via selector matmul: esel2[:, h-block].T @ x0block.
                    # Evacuations alternate ScalarE/VectorE so neither FIFO
                    # gates the PE through the 2-slot bps pool.
                    c0 = st * ST + sb_i * SUB
                    bps = ppbc.tile([U, SUB], F32, tag="bcps", name="bps")
                    nc.tensor.matmul(
                        bps[:], esel2[:, h * U:(h + 1) * U],
                        x0st3[0:F, c0:c0 + SUB],
                        start=True, stop=True,
                    )
                    dst = slab[:, i, sb_i * SUB:(sb_i + 1) * SUB]
                    if sb_i % 2 == 0:
                        nc.scalar.activation(dst, bps[:], AF.Identity)
                    else:
                        nc.vector.tensor_copy(dst, bps[:])

                # Slab groups are created in strict consumption order with a
                # fixed look-ahead of 2 groups (matches pool bufs=3), whether
                # DMA'd or PE-built. This keeps slot-allocation order equal to
                # consumption order (no FIFO deadlock) and lets slab DMA
                # transfers overlap the PE-built group windows.
                bc1g: dict = {}
                bc2g: dict = {}
                pe1_pieces: dict = {}
                pe_pieces: dict = {}
                dma_rr = [0]

                def ensure_bc1(g):
                    if g >= ng1 or g in bc1g:
                        return
                    k0 = g * 4
                    kcnt = min(4, KG1 - k0)
                    if g in pe1:
                        slab = bc1p.tile([117, 4, ST], dt_e, tag="bc1",
                                         name="bc1pe")
                        pe1_pieces[g] = [(i, sb) for i in range(kcnt)
                                         for sb in range(NSUB)]
                    else:
                        slab = bc1p.tile([117, 4, ST], dt_e, tag="bc1",
                                         name="bc1s")
                        for j in range(3):
                            src = x0q1_d[st, j, k0:k0 + kcnt, :] \
                                .partition_broadcast(F)
                            eng = nc.sync if dma_rr[0] % 2 == 0 else nc.scalar
                            eng.dma_start(
                                slab[j * F:(j + 1) * F, 0:kcnt, :], src)
                        dma_rr[0] += 1
                    bc1g[g] = slab

                def ensure_bc2(g):
                    if g >= ng2 or g in bc2g:
                        return
                    h0 = g * 4
                    hcnt = min(4, F - h0)
                    if g in pe2:
                        slab = bc2p.tile([U, 4, ST], dt_e, tag="bc2",
                                         name="bc2pe")
                        pe_pieces[g] = [(i, sb) for i in range(hcnt)
                                        for sb in range(NSUB)]
                    else:
                        slab = bc2p.tile([U, 4, ST], dt_e, tag="bc2",
                                         name="bc2s")
                        src = x0q2_d[st, h0:h0 + hcnt, :].partition_broadcast(U)
                        eng = nc.sync if dma_rr[0] % 2 == 0 else nc.scalar
                        eng.dma_start(slab[:, 0:hcnt, :], src)
                        dma_rr[0] += 1
                    bc2g[g] = slab

                def gen_slab_pe1(slab, k, i, sb_i):
                    # one selector matmul builds the whole 117-row chunk:
                    # out[p] = X0[3k + p//39]
                    c0 = st * ST + sb_i * SUB
                    bps = ppbc.tile([117, SUB], F32, tag="bcps", name="bps1")
                    nc.tensor.matmul(
                        bps[:], esel1[:, k * 117:(k + 1) * 117],
                        x0st3[0:F, c0:c0 + SUB],
                        start=True, stop=True,
                    )
                    dst = slab[:, i, sb_i * SUB:(sb_i + 1) * SUB]
                    if sb_i % 2 == 0:
                        nc.scalar.activation(dst, bps[:], AF.Identity)
                    else:
                        nc.vector.tensor_copy(dst, bps[:])

                def bc1_drip(k):
                    ensure_bc1(k // 4 + 2)
                    for g in sorted(pe1_pieces):
                        lst = pe1_pieces[g]
                        if lst:
                            for _ in range(4):
                                if not lst:
                                    break
                                i, sb_i2 = lst.pop(0)
                                gen_slab_pe1(bc1g[g], g * 4 + i, i, sb_i2)
                            break

                def bc2_drip(h):
                    ensure_bc2(h // 4 + 2)
                    for g in sorted(pe_pieces):
                        lst = pe_pieces[g]
                        if lst:
                            for _ in range(4):
                                if not lst:
                                    break
                                i, sb_i2 = lst.pop(0)
                                gen_slab_pe2(bc2g[g], g * 4 + i, i, sb_i2)
                            break

                ensure_bc1(0)
                ensure_bc1(1)
                ensure_bc2(0)
                ensure_bc2(1)

                # ---- layer 1: X1 = W1 @ (X0 (x) X0) + b1 ----
                x1ps = [ppc.tile([U, SUB], F32, tag="conv", name=f"x1ps{i}")
                         for i in range(NSUB)]
                for k in range(KG1):
                    had1 = hadp.tile([117, ST], dt_e, tag="had1")
                    nc.vector.tensor_mul(
                        had1[:], x0st3[:, cols], bc1g[k // 4][:, k % 4, :])
                    for sb_i in range(NSUB):
                        nc.tensor.matmul(
                            x1ps[sb_i][:], w1sb[:, k * U:(k + 1) * U],
                            had1[:, sb_i * SUB:(sb_i + 1) * SUB],
                            start=(k == 0), stop=(k == KG1 - 1),
                        )
                    emit_l3()     # drip previous super-tile's layer-3 work
                    bc1_drip(k)
                x1sb = xsbp.tile([U, ST], dt_e, tag="x1")
                for sb_i in range(NSUB):
                    dst = x1sb[:, sb_i * SUB:(sb_i + 1) * SUB]
                    if sb_i % 2 == 0:
                        nc.scalar.activation(dst, x1ps[sb_i][:],
                                             AF.Identity, bias=b1sb[:], scale=1.0)
                    else:
                        nc.vector.scalar_tensor_tensor(
                            dst, x1ps[sb_i][:], 1.0, b1bc[:],
                            mybir.AluOpType.mult, mybir.AluOpType.add)
                # pooled1 contribution: strided d-sum on DVE, dripped later
                # so it never sits between the layer-1 evacs and layer-2 TTs
                def red1(st=st, x1sb=x1sb):
                    nc.vector.tensor_reduce(
                        pooled1[:, st * (ST // D):(st + 1) * (ST // D)],
                        x1sb[:].rearrange("p (b d) -> p b d", d=D),
                        mybir.AxisListType.X, mybir.AluOpType.add)
                l3_pending.append(red1)

                # ---- layer 2: X2 = W2 @ (X0 (x) X1) + b2 ----
                x2ps = [ppc.tile([U, SUB], F32, tag="conv", name=f"x2ps{i}")
                         for i in range(NSUB)]
                for h in range(F):
                    had2 = hadp.tile([U, ST], dt_e, tag="had2")
                    nc.vector.tensor_mul(
                        had2[:], x1sb[:], bc2g[h // 4][:, h % 4, :])
                    for sb_i in range(NSUB):
                        nc.tensor.matmul(
                            x2ps[sb_i][:], w2sb[:, h * U:(h + 1) * U],
                            had2[:, sb_i * SUB:(sb_i + 1) * SUB],
                            start=(h == 0), stop=(h == F - 1),
                        )
                    emit_l3()
                    bc2_drip(h)
                x2sb = xsbp.tile([U, ST], dt_e, tag="x2")
                for sb_i in range(NSUB):
                    dst = x2sb[:, sb_i * SUB:(sb_i + 1) * SUB]
                    if sb_i % 2 == 0:
                        nc.scalar.activation(dst, x2ps[sb_i][:],
                                             AF.Identity, bias=b2sb[:], scale=1.0)
                    else:
                        nc.vector.scalar_tensor_tensor(
                            dst, x2ps[sb_i][:], 1.0, b2bc[:],
                            mybir.AluOpType.mult, mybir.AluOpType.add)
                def red2(st=st, x2sb=x2sb):
                    nc.vector.tensor_reduce(
                        pooled2[:, st * (ST // D):(st + 1) * (ST // D)],
                        x2sb[:].rearrange("p (b d) -> p b d", d=D),
                        mybir.AxisListType.X, mybir.AluOpType.add)
                l3_pending.append(red2)

                # ---- layer 3 Gram: G2[m, h, b] = sum_d X2[m,bd] X0[h,bd] ----
                # queue as lazily-emitted pieces, dripped into the next
                # super-tile's conv loops so the PE never starves (HAM warm)
                def queue_l3(st=st, x2sb=x2sb):
                    for s2 in range(ST // D // 2):       # 2 samples / transpose
                        def piece(s2=s2, st=st, x2sb=x2sb):
                            x2t_ps = pptg.tile([U, U], dt_e, tag="tg",
                                               name="x2tps")
                            nc.tensor.transpose(
                                x2t_ps[:],
                                x2sb[:, s2 * 2 * D:(s2 + 1) * 2 * D], id16[:])
                            x2t = l3p.tile([U, U], dt_e, tag="x2t", name="x2t")
                            nc.scalar.activation(x2t[:], x2t_ps[:], AF.Identity)
                            for ls in range(2):
                                b = st * (ST // D) + s2 * 2 + ls
                                g2ps = pptg.tile([U, F], F32, tag="tg",
                                                 name="g2ps")
                                nc.tensor.matmul(
                                    g2ps[:], x2t[ls * D:(ls + 1) * D, :],
                                    x0dt[ls * D:(ls + 1) * D,
                                         b * F:(b + 1) * F],
                                    start=True, stop=True,
                                )
                                nc.scalar.activation(
                                    g2f[:, :, b], g2ps[:], AF.Identity)
                        l3_pending.append(piece)
                queue_l3()
            while l3_pending:     # flush the last super-tile's layer-3 work
                l3_pending.pop(0)()

            # ---- pooled3 = W3 @ G2 + 64*b3 ----
            if True:
                p3ps = ppbc.tile([U, BC], F32, tag="bcps", name="p3ps")
                for h in range(F):
                    nc.tensor.matmul(
                        p3ps[:], w3sb[:, h * U:(h + 1) * U], g2f[:, h, :],
                        start=(h == 0), stop=(h == F - 1),
                    )
                nc.scalar.activation(
                    pooled3[:], p3ps[:], AF.Identity, bias=b3sb[:], scale=1.0)

                # ---- transpose pooled_i -> [b, o] and store ----
                for i, pl in enumerate((pooled1, pooled2, pooled3)):
                    trp = ppbc.tile([BC, U], F32, tag="bcps", name="trp")
                    nc.tensor.transpose(trp[:], pl[:], id32[:])
                    nc.scalar.activation(
                        outsb[:, i * U:(i + 1) * U], trp[:], AF.Identity)
                nc.sync.dma_start(y_d[:, :], outsb[:])

    nc.compile()
    return nc


def _prep_in_maps(inputs, np_e):
    X0 = np.asarray(inputs["X_0"], np.float32)
    W1 = np.asarray(inputs["W1"], np.float32)
    b1 = np.asarray(inputs["b1"], np.float32)
    W2 = np.asarray(inputs["W2"], np.float32)
    b2 = np.asarray(inputs["b2"], np.float32)
    W3 = np.asarray(inputs["W3"], np.float32)
    b3 = np.asarray(inputs["b3"], np.float32)

    # W1 reorder: [117, 13*128]; rows p=j*39+m, cols k*128+o -> W1[o, (3k+j)*39+m]
    w1r = W1.reshape(U, F, F)                    # [o, h, m]
    w1p = np.zeros((117, KG1 * U), np.float32)
    for k in range(KG1):
        for j in range(3):
            w1p[j * F:(j + 1) * F, k * U:(k + 1) * U] = w1r[:, 3 * k + j, :].T
    # [m, h*128+o]
    w2p = W2.reshape(U, F, U).transpose(2, 1, 0).reshape(U, F * U)
    w3p = W3.reshape(U, F, U).transpose(2, 1, 0).reshape(U, F * U)

    # selector matrices for PE-side broadcast generation
    es1 = np.zeros((F, KG1 * 117), np.float32)
    for k in range(KG1):
        for p in range(117):
            es1[3 * k + p // F, k * 117 + p] = 1.0
    es2 = np.zeros((F, F * U), np.float32)
    for h in range(F):
        es2[h, h * U:(h + 1) * U] = 1.0
    es1 = es1.astype(np_e)
    es2 = es2.astype(np_e)

    shared = {
        "w1p": w1p.astype(np_e),
        "w2p": w2p.astype(np_e),
        "w3p": w3p.astype(np_e),
        "b1c": b1.reshape(U, 1).astype(np.float32),
        "b2c": b2.reshape(U, 1).astype(np.float32),
        "b3c": (D * b3).reshape(U, 1).astype(np.float32),
        "id16": np.eye(U, dtype=np_e),
        "id32": np.eye(U, dtype=np.float32),
    }
    in_maps = []
    for c in range(NCORES):
        xs = X0[c * BC:(c + 1) * BC]                         # [128, 39, 64]
        x0cp = xs.transpose(1, 0, 2).reshape(F, BD)          # [h, b*64+d]
        x0dt = xs.transpose(2, 0, 1).reshape(D, BC * F)      # [d, b*39+h]
        x0st = x0cp.reshape(F, NST, ST)
        x0q2 = x0st.transpose(1, 0, 2)                       # [st, h, c]
        x0q1 = np.zeros((NST, 3, KG1, ST), np.float32)
        for j in range(3):
            for k in range(KG1):
                x0q1[:, j, k, :] = x0st[3 * k + j].reshape(NST, ST)
        m = dict(shared)
        m["x0cp"] = x0cp.astype(np_e)
        m["x0dt"] = x0dt.astype(np_e)
        m["x0q1"] = np.ascontiguousarray(x0q1).astype(np_e)
        m["x0q2"] = np.ascontiguousarray(x0q2).astype(np_e)
        m["esel1"] = es1
        m["esel2"] = es2
        in_maps.append(m)
    return in_maps


def _run(inputs, trace=False, **kw):
    dt_e, np_e = _dtype_cfg()
    offg2, offg1 = _off_cfg()
    key = (dt_e, offg2, offg1)
    if key not in _CACHE:
        _CACHE[key] = _build(dt_e, offg2, offg1)
    nc = _CACHE[key]
    in_maps = _prep_in_maps(inputs, np_e)
    res = bass_utils.run_bass_kernel_spmd(
        nc, in_maps, core_ids=list(range(NCORES)), trace=trace, **kw)
    y = np.concatenate([r["y"] for r in res.results], axis=0).astype(np.float32)
    return y, res


def kernel(**inputs) -> np.ndarray:
    y, _ = _run(inputs, trace=False)
    return y
